# revision 3
# baseline (speedup 1.0000x reference)
"""Interference self-attention Trainium2 kernel.

Head-sharded SPMD over 8 NeuronCores: core i owns heads {2i, 2i+1}
(= model dims [128i, 128i+128)).

Math (per head): q_amp = softplus(x@Wqa.T), q_phi = pi*tanh(x@Wqp.T) (same for k),
v = x@Wv.T, scores = (qa*cos(qp))@(ka*cos(kp)).T + (qa*sin(qp))@(ka*sin(kp)).T
                   = uq @ uk.T  with  uq = [qa*cos(qp); qa*sin(qp)]  (128-dim)
out = softmax_causal(scores/8) @ v ; y = merge_heads(out) @ Wo.T

Per-core pipeline (all bf16 matmul inputs, f32 PSUM accumulation):
  stage 1a: phi projections -> tanh -> sin/cos          (ACT set: silu_and_others)
  stage 1b: amp projections -> exp -> ln(1+e) [softplus], u = amp*{cos,sin};
            v projection (normal layout, ones-row appended for softmax sums)
                                                        (ACT set: natural_log_exp)
  stage 2:  per (batch, head, 512-q-block): scoresT tiles [128k x 512q] on PE,
            exp (ACT, scale=1/8), causal 0/1 mask on diagonal tiles,
            attn@v via v_aug -> psum [65, 512] (row 64 = softmax sums),
            normalize, stage to AllToAll input.
  AllToAll (one per batch): redistribute head-dims -> row-chunks.
  stage 3:  y rows = mergedT.T @ Wo.T for this core's 512 rows.

Host side only reshapes/transposes/casts and concatenates per-core outputs.
"""

import math

import numpy as np
import ml_dtypes

B, S, D, H = 2, 2048, 1024, 16
HD = D // H            # 64
N = B * S              # 4096 rows
NCORES = 8
HPC = H // NCORES      # 2 heads per core
DPC = D // NCORES      # 128 dims per core
ROWS_PC = N // NCORES  # 512 rows per core (256 per batch)

BF16 = ml_dtypes.bfloat16

_CACHE = {}


def _build_program():
    import concourse.bass as bass
    import concourse.mybir as mybir
    import concourse.tile as tile
    from concourse import bacc

    f32 = mybir.dt.float32
    bf16 = mybir.dt.bfloat16
    AFT = mybir.ActivationFunctionType

    nc = bacc.Bacc(
        "TRN2",
        target_bir_lowering=False,
        debug=False,
        num_devices=NCORES,
    )

    # ---- I/O -------------------------------------------------------------
    xT = nc.dram_tensor("xT", [D, N], bf16, kind="ExternalInput")
    wqaT = nc.dram_tensor("wqaT", [D, DPC], bf16, kind="ExternalInput")
    wkaT = nc.dram_tensor("wkaT", [D, DPC], bf16, kind="ExternalInput")
    wqpT = nc.dram_tensor("wqpT", [D, DPC], bf16, kind="ExternalInput")
    wkpT = nc.dram_tensor("wkpT", [D, DPC], bf16, kind="ExternalInput")
    wvT = nc.dram_tensor("wvT", [D, DPC], bf16, kind="ExternalInput")
    woT = nc.dram_tensor("woT", [8, 128, D], bf16, kind="ExternalInput")
    masks = nc.dram_tensor("masks", [4, 128, 512], bf16, kind="ExternalInput")
    y = nc.dram_tensor("y", [B, 2, 128, D], f32, kind="ExternalOutput")

    NT = 4          # 1024-column tiles over N
    NTW = N // NT   # 1024
    KC = 8          # 128-row contraction chunks over D

    with tile.TileContext(nc) as tc:
        with (
            tc.tile_pool(name="persist", bufs=1) as pp,
            tc.tile_pool(name="dram", bufs=1, space="DRAM") as dp,
        ):
            # persistent SBUF tensors
            uq = [pp.tile([128, N], bf16, name=f"uq{h}") for h in range(HPC)]
            uk = [pp.tile([128, N], bf16, name=f"uk{h}") for h in range(HPC)]
            v_aug = pp.tile([128, N // 128, 130], bf16, name="v_aug")
            w_sbs = {}
            for nm, t in (("wqa", wqaT), ("wka", wkaT), ("wqp", wqpT),
                          ("wkp", wkpT), ("wv", wvT)):
                sb = pp.tile([128, KC, DPC], bf16, name=f"{nm}_sb")
                nc.sync.dma_start(
                    out=sb, in_=t[:, :].rearrange("(c p) m -> p c m", p=128))
                w_sbs[nm] = sb
            wo_sb = pp.tile([128, KC, D], bf16, name="wo_sb")
            nc.sync.dma_start(out=wo_sb, in_=woT[:, :, :].transpose([1, 0, 2]))
            mask_sb = pp.tile([128, 4, 512], bf16, name="mask_sb")
            nc.sync.dma_start(out=mask_sb, in_=masks[:, :, :].transpose([1, 0, 2]))

            # ones columns of v_aug (col 64 for head 0, col 129 for head 1)
            nc.vector.memset(v_aug[:, :, 64], 1.0)
            nc.vector.memset(v_aug[:, :, 129], 1.0)

            # AllToAll bounce buffers (one pair per batch)
            a2a_in = [dp.tile([8, 128, 256], bf16, name=f"a2a_in{b}")
                      for b in range(B)]
            a2a_out = [dp.tile([8, 128, 256], bf16, name=f"a2a_out{b}")
                       for b in range(B)]

            # ---- stage 1: projections -----------------------------------
            with (
                tc.tile_pool(name="xt", bufs=2) as xp,
                tc.tile_pool(name="cossin", bufs=1) as cp,
                tc.tile_pool(name="s1tmp", bufs=3) as tp,
                tc.tile_pool(name="ps1", bufs=3, space="PSUM") as ps1,
                tc.tile_pool(name="psv", bufs=2, space="PSUM") as psv,
            ):
                cos_sb = {t: cp.tile([128, N], bf16, name=f"cos_{t}")
                          for t in ("q", "k")}
                sin_sb = {t: cp.tile([128, N], bf16, name=f"sin_{t}")
                          for t in ("q", "k")}

                def proj_psum(w_sb, xa):
                    ps = ps1.tile([128, NTW], mybir.dt.float32, tag="proj")
                    for half in range(2):
                        cols = slice(512 * half, 512 * half + 512)
                        for c in range(KC):
                            nc.tensor.matmul(
                                ps[:, cols],
                                w_sb[:, c, :],
                                xa[:, c, cols],
                                start=(c == 0),
                                stop=(c == KC - 1),
                            )
                    return ps

                def load_xa(nt):
                    xa = xp.tile([128, KC, NTW], bf16, tag="xa")
                    nc.sync.dma_start(
                        out=xa,
                        in_=xT[:, NTW * nt:NTW * (nt + 1)].rearrange(
                            "(c p) m -> p c m", p=128),
                    )
                    return xa

                # phase A: phi -> tanh -> sin/cos   (silu_and_others table)
                for nt in range(NT):
                    xa = load_xa(nt)
                    csl = slice(NTW * nt, NTW * (nt + 1))
                    for t in ("q", "k"):
                        ps = proj_psum(w_sbs[f"w{t}p"], xa)
                        t_sb = tp.tile([128, NTW], f32, tag="tanh")
                        nc.scalar.activation(t_sb, ps, AFT.Tanh)
                        w_sb = tp.tile([128, NTW], f32, tag="wrap")
                        nc.vector.add_range_wrap(w_sb, t_sb, 0.5, 1.0, 2.0)
                        nc.scalar.activation(
                            sin_sb[t][:, csl], t_sb, AFT.Sin, scale=math.pi)
                        nc.scalar.activation(
                            cos_sb[t][:, csl], w_sb, AFT.Sin, scale=math.pi)

                # phase B: amp -> softplus, u-muls; v projection
                # (natural_log_exp_and_others table)
                u_of = {"q": uq, "k": uk}
                for nt in range(NT):
                    xa = load_xa(nt)
                    csl = slice(NTW * nt, NTW * (nt + 1))
                    for t in ("q", "k"):
                        ps = proj_psum(w_sbs[f"w{t}a"], xa)
                        e_sb = tp.tile([128, NTW], f32, tag="esb")
                        nc.scalar.activation(e_sb, ps, AFT.Exp)
                        amp = tp.tile([128, NTW], bf16, tag="amp")
                        nc.scalar.activation(amp, e_sb, AFT.Ln, bias=1.0)
                        for h in range(HPC):
                            hsl = slice(64 * h, 64 * h + 64)
                            nc.vector.tensor_mul(
                                u_of[t][h][0:64, csl], amp[hsl, :],
                                cos_sb[t][hsl, csl])
                            nc.vector.tensor_mul(
                                u_of[t][h][64:128, csl], amp[hsl, :],
                                sin_sb[t][hsl, csl])
                    # v projection for this nt (normal layout + ones cols)
                    for stl in range(8):
                        st = 8 * nt + stl
                        vps = psv.tile([128, 128], f32, tag="vps")
                        for c in range(KC):
                            nc.tensor.matmul(
                                vps,
                                xa[:, c, 128 * stl:128 * stl + 128],
                                w_sbs["wv"][:, c, :],
                                start=(c == 0),
                                stop=(c == KC - 1),
                            )
                        nc.vector.tensor_copy(
                            v_aug[:, st, :].rearrange(
                                "p (g c) -> p g c", g=2)[:, :, 0:64],
                            vps[:, :].rearrange("p (g c) -> p g c", g=2),
                        )

            # ---- stage 2 + 3 --------------------------------------------
            with (
                tc.tile_pool(name="exp", bufs=2) as ep,
                tc.tile_pool(name="s2tmp", bufs=3) as sp,
                tc.tile_pool(name="mout", bufs=2) as mp,
                tc.tile_pool(name="ps2", bufs=4, space="PSUM") as ps2,
                tc.tile_pool(name="pso", bufs=2, space="PSUM") as pso,
            ):
                def stage3(b):
                    m_sb = mp.tile([128, KC, 256], bf16, tag="m")
                    nc.sync.dma_start(
                        out=m_sb, in_=a2a_out[b][:, :, :].transpose([1, 0, 2]))
                    for rt in range(2):
                        y_sb = mp.tile([128, D], f32, tag="ysb")
                        for half in range(2):
                            fps = ps2.tile([128, 512], f32, tag="s")
                            for c in range(KC):
                                nc.tensor.matmul(
                                    fps,
                                    m_sb[:, c, 128 * rt:128 * rt + 128],
                                    wo_sb[:, c, 512 * half:512 * half + 512],
                                    start=(c == 0),
                                    stop=(c == KC - 1),
                                )
                            nc.vector.tensor_copy(
                                y_sb[:, 512 * half:512 * half + 512], fps)
                        nc.sync.dma_start(out=y[b, rt], in_=y_sb)

                for b in range(B):
                    for h in range(HPC):
                        for qb in range(4):
                            qsl = slice(2048 * b + 512 * qb,
                                        2048 * b + 512 * qb + 512)
                            njk = 4 * qb + 4
                            exp_t = ep.tile([128, 16, 512], bf16, tag="exp")
                            for kt in range(njk):
                                ksl = slice(2048 * b + 128 * kt,
                                            2048 * b + 128 * kt + 128)
                                s_ps = ps2.tile([128, 512], f32, tag="s")
                                nc.tensor.matmul(
                                    s_ps, uk[h][:, ksl], uq[h][:, qsl],
                                    start=True, stop=True)
                                nc.scalar.activation(
                                    exp_t[:, kt, :], s_ps, AFT.Exp,
                                    scale=0.125)
                                if kt >= 4 * qb:
                                    nc.vector.tensor_mul(
                                        exp_t[:, kt, :], exp_t[:, kt, :],
                                        mask_sb[:, kt - 4 * qb, :])
                            o_ps = pso.tile([65, 512], f32, tag="o")
                            for kt in range(njk):
                                nc.tensor.matmul(
                                    o_ps,
                                    v_aug[:, 16 * b + kt, 65 * h:65 * h + 65],
                                    exp_t[:, kt, :],
                                    start=(kt == 0),
                                    stop=(kt == njk - 1),
                                )
                            recip = sp.tile([1, 512], f32, tag="recip")
                            nc.vector.reciprocal(recip, o_ps[64:65, :])
                            bc = sp.tile([64, 512], f32, tag="bc")
                            nc.gpsimd.partition_broadcast(bc, recip)
                            outn = sp.tile([64, 512], bf16, tag="outn")
                            nc.vector.tensor_mul(outn, o_ps[0:64, :], bc)
                            nc.sync.dma_start(
                                out=a2a_in[b][2 * qb:2 * qb + 2,
                                              64 * h:64 * h + 64,
                                              :].transpose([1, 0, 2]),
                                in_=outn[:, :].rearrange(
                                    "p (c m) -> p c m", c=2),
                            )
                    nc.gpsimd.collective_compute(
                        "AllToAll",
                        mybir.AluOpType.bypass,
                        replica_groups=[list(range(NCORES))],
                        ins=[a2a_in[b][:, :, :].opt()],
                        outs=[a2a_out[b][:, :, :].opt()],
                    )
                    stage3(b)

    nc.compile()
    return nc


def _host_inputs(x, wq_amp, wk_amp, wq_phi, wk_phi, wv, wo):
    """Per-core input maps: pure layout transforms + dtype casts."""
    xT = np.ascontiguousarray(
        x.reshape(N, D).T).astype(BF16)
    woT = np.ascontiguousarray(wo.T).reshape(8, 128, D).astype(BF16)
    # causal 0/1 masks for the 4 diagonal k-tile offsets
    kl = np.arange(128)[:, None]
    ql = np.arange(512)[None, :]
    masks = np.stack(
        [(ql >= kl + 128 * p) for p in range(4)]).astype(BF16)
    in_maps = []
    for i in range(NCORES):
        rsl = slice(DPC * i, DPC * (i + 1))
        in_maps.append({
            "xT": xT,
            "wqaT": np.ascontiguousarray(wq_amp[rsl].T).astype(BF16),
            "wkaT": np.ascontiguousarray(wk_amp[rsl].T).astype(BF16),
            "wqpT": np.ascontiguousarray(wq_phi[rsl].T).astype(BF16),
            "wkpT": np.ascontiguousarray(wk_phi[rsl].T).astype(BF16),
            "wvT": np.ascontiguousarray(wv[rsl].T).astype(BF16),
            "woT": woT,
            "masks": masks,
        })
    return in_maps


def _assemble(results):
    out = np.empty((B, S, D), np.float32)
    for i, r in enumerate(results):
        yc = r["y"].reshape(B, 256, D)
        for b in range(B):
            out[b, 256 * i:256 * (i + 1), :] = yc[b]
    return out


def run(inputs, trace=False):
    from concourse.bass_utils import run_bass_kernel_spmd

    if "nc" not in _CACHE:
        _CACHE["nc"] = _build_program()
    nc = _CACHE["nc"]
    in_maps = _host_inputs(**inputs)
    res = run_bass_kernel_spmd(
        nc, in_maps, list(range(NCORES)), trace=trace)
    return _assemble(res.results), res


def kernel(x, wq_amp, wk_amp, wq_phi, wk_phi, wv, wo):
    out, _ = run(dict(x=x, wq_amp=wq_amp, wk_amp=wk_amp, wq_phi=wq_phi,
                      wk_phi=wk_phi, wv=wv, wo=wo))
    return out


# revision 11
# speedup vs baseline: 1.2133x; 1.2133x over previous
"""Interference self-attention Trainium2 kernel.

Head-sharded SPMD over 8 NeuronCores: core i owns heads {2i, 2i+1}
(= model dims [128i, 128i+128)).

Math (per head): q_amp = softplus(x@Wqa.T), q_phi = pi*tanh(x@Wqp.T) (same for k),
v = x@Wv.T, scores = (qa*cos(qp))@(ka*cos(kp)).T + (qa*sin(qp))@(ka*sin(kp)).T
                   = uq @ uk.T  with  uq = [qa*cos(qp); qa*sin(qp)]  (128-dim)
out = softmax_causal(scores/8) @ v ; y = merge_heads(out) @ Wo.T

Per-core pipeline (all bf16 matmul inputs, f32 PSUM accumulation):
  stage 1a: phi projections -> tanh -> sin/cos          (ACT set: silu_and_others)
  stage 1b: amp projections -> exp -> ln(1+e) [softplus], u = amp*{cos,sin};
            v projection (normal layout, ones-row appended for softmax sums)
                                                        (ACT set: natural_log_exp)
  stage 2:  per (batch, head, 512-q-block): scoresT tiles [128k x 512q] on PE,
            exp (ACT, scale=1/8), causal 0/1 mask on diagonal tiles,
            attn@v via v_aug -> psum [65, 512] (row 64 = softmax sums),
            normalize, stage to AllToAll input.
  AllToAll (one per batch): redistribute head-dims -> row-chunks.
  stage 3:  y rows = mergedT.T @ Wo.T for this core's 512 rows.

Host side only reshapes/transposes/casts and concatenates per-core outputs.
"""

import math

import numpy as np
import ml_dtypes

B, S, D, H = 2, 2048, 1024, 16
HD = D // H            # 64
N = B * S              # 4096 rows
NCORES = 8
HPC = H // NCORES      # 2 heads per core
DPC = D // NCORES      # 128 dims per core
ROWS_PC = N // NCORES  # 512 rows per core (256 per batch)

BF16 = ml_dtypes.bfloat16

_CACHE = {}


def _build_program():
    import concourse.bass as bass
    import concourse.mybir as mybir
    import concourse.tile as tile
    from concourse import bacc

    f32 = mybir.dt.float32
    bf16 = mybir.dt.bfloat16
    AFT = mybir.ActivationFunctionType

    nc = bacc.Bacc(
        "TRN2",
        target_bir_lowering=False,
        debug=False,
        num_devices=NCORES,
    )

    # ---- I/O -------------------------------------------------------------
    xT = nc.dram_tensor("xT", [D, N], bf16, kind="ExternalInput")
    wqaT = nc.dram_tensor("wqaT", [D, DPC], bf16, kind="ExternalInput")
    wkaT = nc.dram_tensor("wkaT", [D, DPC], bf16, kind="ExternalInput")
    wqpT = nc.dram_tensor("wqpT", [D, DPC], bf16, kind="ExternalInput")
    wkpT = nc.dram_tensor("wkpT", [D, DPC], bf16, kind="ExternalInput")
    wvT = nc.dram_tensor("wvT", [D, DPC], bf16, kind="ExternalInput")
    woT = nc.dram_tensor("woT", [8, 128, D], bf16, kind="ExternalInput")
    masks = nc.dram_tensor("masks", [4, 128, 512], bf16, kind="ExternalInput")
    y = nc.dram_tensor("y", [B, 2, 128, D], f32, kind="ExternalOutput")

    NT = 4          # 1024-column tiles over N
    NTW = N // NT   # 1024
    KC = 8          # 128-row contraction chunks over D

    with tile.TileContext(nc) as tc:
        with (
            tc.tile_pool(name="persist", bufs=1) as pp,
            tc.tile_pool(name="dram", bufs=1, space="DRAM") as dp,
        ):
            # persistent SBUF tensors
            uq = [pp.tile([128, N], bf16, name=f"uq{h}") for h in range(HPC)]
            uk = [pp.tile([128, N], bf16, name=f"uk{h}") for h in range(HPC)]
            v_aug = pp.tile([128, N // 128, 130], bf16, name="v_aug")
            # phi-projection weights first: they gate the very first matmuls.
            w_sbs = {}
            for nm, t in (("wqp", wqpT), ("wkp", wkpT), ("wqa", wqaT),
                          ("wka", wkaT), ("wv", wvT)):
                sb = pp.tile([128, KC, DPC], bf16, name=f"{nm}_sb")
                nc.sync.dma_start(
                    out=sb, in_=t[:, :].rearrange("(c p) m -> p c m", p=128))
                w_sbs[nm] = sb
            wo_sb = pp.tile([128, KC, D], bf16, name="wo_sb")
            mask_sb = pp.tile([128, 4, 512], bf16, name="mask_sb")

            # ones columns of v_aug (col 64 for head 0, col 129 for head 1)
            nc.vector.memset(v_aug[:, :, 64], 1.0)
            nc.vector.memset(v_aug[:, :, 129], 1.0)

            # AllToAll bounce buffers (one pair per batch)
            a2a_in = [dp.tile([8, 128, 256], bf16, name=f"a2a_in{b}")
                      for b in range(B)]
            a2a_out = [dp.tile([8, 128, 256], bf16, name=f"a2a_out{b}")
                       for b in range(B)]

            # ---- stage 1: projections -----------------------------------
            with (
                tc.tile_pool(name="xt", bufs=2) as xp,
                tc.tile_pool(name="cossin", bufs=1) as cp,
                tc.tile_pool(name="s1tmp", bufs=2) as tp,
                tc.tile_pool(name="ps1", bufs=3, space="PSUM") as ps1,
                tc.tile_pool(name="psv", bufs=2, space="PSUM") as psv,
            ):
                cos_sb = {t: cp.tile([128, N], bf16, name=f"cos_{t}")
                          for t in ("q", "k")}
                sin_sb = {t: cp.tile([128, N], bf16, name=f"sin_{t}")
                          for t in ("q", "k")}

                def proj_psum(w_sb, xa, name):
                    ps = ps1.tile([128, NTW], mybir.dt.float32, tag="proj", name=name)
                    for half in range(2):
                        cols = slice(512 * half, 512 * half + 512)
                        for c in range(KC):
                            nc.tensor.matmul(
                                ps[:, cols],
                                w_sb[:, c, :],
                                xa[:, c, cols],
                                start=(c == 0),
                                stop=(c == KC - 1),
                            )
                    return ps

                def load_xa(nt):
                    # per-k-chunk DMAs so the first matmul starts after 256KB
                    xa = xp.tile([128, KC, NTW], bf16, tag="xa")
                    src = xT[:, NTW * nt:NTW * (nt + 1)].rearrange(
                        "(c p) m -> p c m", p=128)
                    for c in range(KC):
                        nc.sync.dma_start(out=xa[:, c, :], in_=src[:, c, :])
                    return xa

                # phase A: phi -> tanh -> sin/cos. q/k activations paired so
                # the ACT queue runs [tanh,tanh][sin,sin,sin,sin] per nt
                # (2 table loads instead of 4).
                for nt in range(NT):
                    xa = load_xa(nt)
                    csl = slice(NTW * nt, NTW * (nt + 1))
                    ps_of, t_of, w_of = {}, {}, {}
                    for t in ("q", "k"):
                        ps_of[t] = proj_psum(w_sbs[f"w{t}p"], xa, f"psp_{t}_{nt}")
                    for t in ("q", "k"):
                        t_of[t] = tp.tile([128, NTW], f32, tag=f"tanh{t}", name=f"t_{t}_{nt}")
                        nc.scalar.activation(t_of[t], ps_of[t], AFT.Tanh)
                    for t in ("q", "k"):
                        w_of[t] = tp.tile([128, NTW], f32, tag=f"wrap{t}", name=f"w_{t}_{nt}")
                        nc.vector.add_range_wrap(w_of[t], t_of[t], 0.5, 1.0, 2.0)
                    for t in ("q", "k"):
                        nc.scalar.activation(
                            sin_sb[t][:, csl], t_of[t], AFT.Sin, scale=math.pi)
                        nc.scalar.activation(
                            cos_sb[t][:, csl], w_of[t], AFT.Sin, scale=math.pi)

                # phase B: amp -> softplus (exp then ln, q/k paired), u-muls;
                # v projection
                u_of = {"q": uq, "k": uk}
                for nt in range(NT):
                    xa = load_xa(nt)
                    csl = slice(NTW * nt, NTW * (nt + 1))
                    ps_of, e_of, a_of = {}, {}, {}
                    for t in ("q", "k"):
                        ps_of[t] = proj_psum(w_sbs[f"w{t}a"], xa, f"psa_{t}_{nt}")
                    for t in ("q", "k"):
                        e_of[t] = tp.tile([128, NTW], bf16, tag=f"esb{t}", name=f"e_{t}_{nt}")
                        nc.scalar.activation(e_of[t], ps_of[t], AFT.Exp)
                    for t in ("q", "k"):
                        a_of[t] = tp.tile([128, NTW], bf16, tag=f"amp{t}", name=f"a_{t}_{nt}")
                        nc.scalar.activation(a_of[t], e_of[t], AFT.Ln, bias=1.0)
                    for t in ("q", "k"):
                        for h in range(HPC):
                            hsl = slice(64 * h, 64 * h + 64)
                            nc.vector.tensor_mul(
                                u_of[t][h][0:64, csl], a_of[t][hsl, :],
                                cos_sb[t][hsl, csl])
                            nc.vector.tensor_mul(
                                u_of[t][h][64:128, csl], a_of[t][hsl, :],
                                sin_sb[t][hsl, csl])
                    # v projection for this nt (normal layout + ones cols)
                    for stl in range(8):
                        st = 8 * nt + stl
                        vps = psv.tile([128, 128], f32, tag="vps")
                        for c in range(KC):
                            nc.tensor.matmul(
                                vps,
                                xa[:, c, 128 * stl:128 * stl + 128],
                                w_sbs["wv"][:, c, :],
                                start=(c == 0),
                                stop=(c == KC - 1),
                            )
                        nc.vector.tensor_copy(
                            v_aug[:, st, :].rearrange(
                                "p (g c) -> p g c", g=2)[:, :, 0:64],
                            vps[:, :].rearrange("p (g c) -> p g c", g=2),
                        )

            # ---- stage 2 + 3 --------------------------------------------
            nc.sync.dma_start(out=mask_sb, in_=masks[:, :, :].transpose([1, 0, 2]))
            nc.sync.dma_start(out=wo_sb, in_=woT[:, :, :].transpose([1, 0, 2]))
            with (
                tc.tile_pool(name="exp", bufs=3) as ep,
                tc.tile_pool(name="s2tmp", bufs=3) as sp,
                tc.tile_pool(name="mout", bufs=2) as mp,
                tc.tile_pool(name="ps2", bufs=4, space="PSUM") as ps2,
                tc.tile_pool(name="pso", bufs=3, space="PSUM") as pso,
            ):
                def stage3(b):
                    m_sb = mp.tile([128, KC, 256], bf16, tag="m")
                    nc.sync.dma_start(
                        out=m_sb, in_=a2a_out[b][:, :, :].transpose([1, 0, 2]))
                    for rt in range(2):
                        y_sb = mp.tile([128, D], f32, tag="ysb")
                        for half in range(2):
                            fps = ps2.tile([128, 512], f32, tag="s")
                            for c in range(KC):
                                nc.tensor.matmul(
                                    fps,
                                    m_sb[:, c, 128 * rt:128 * rt + 128],
                                    wo_sb[:, c, 512 * half:512 * half + 512],
                                    start=(c == 0),
                                    stop=(c == KC - 1),
                                )
                            nc.vector.tensor_copy(
                                y_sb[:, 512 * half:512 * half + 512], fps)
                        nc.sync.dma_start(out=y[b, rt], in_=y_sb)

                def attention(b):
                    for h in range(HPC):
                        for qb in range(4):
                            qsl = slice(2048 * b + 512 * qb,
                                        2048 * b + 512 * qb + 512)
                            njk = 4 * qb + 4
                            exp_t = ep.tile([128, 16, 512], bf16, tag="exp")
                            for kt in range(njk):
                                ksl = slice(2048 * b + 128 * kt,
                                            2048 * b + 128 * kt + 128)
                                s_ps = ps2.tile([128, 512], f32, tag="s")
                                nc.tensor.matmul(
                                    s_ps, uk[h][:, ksl], uq[h][:, qsl],
                                    start=True, stop=True)
                                nc.scalar.activation(
                                    exp_t[:, kt, :], s_ps, AFT.Exp,
                                    scale=0.125)
                                if kt >= 4 * qb:
                                    nc.vector.tensor_mul(
                                        exp_t[:, kt, :], exp_t[:, kt, :],
                                        mask_sb[:, kt - 4 * qb, :])
                            o_ps = pso.tile([65, 512], f32, tag="o")
                            for kt in range(njk):
                                nc.tensor.matmul(
                                    o_ps,
                                    v_aug[:, 16 * b + kt, 65 * h:65 * h + 65],
                                    exp_t[:, kt, :],
                                    start=(kt == 0),
                                    stop=(kt == njk - 1),
                                )
                            sums = sp.tile([1, 512], f32, tag="sums")
                            nc.vector.tensor_copy(sums, o_ps[64:65, :])
                            recip = sp.tile([1, 512], f32, tag="recip")
                            nc.vector.reciprocal_approx_fast(recip, sums)
                            bc = sp.tile([64, 512], f32, tag="bc")
                            nc.gpsimd.partition_broadcast(bc, recip)
                            outn = sp.tile([64, 512], bf16, tag="outn")
                            nc.vector.tensor_mul(outn, o_ps[0:64, :], bc)
                            nc.sync.dma_start(
                                out=a2a_in[b][2 * qb:2 * qb + 2,
                                              64 * h:64 * h + 64,
                                              :].transpose([1, 0, 2]),
                                in_=outn[:, :].rearrange(
                                    "p (c m) -> p c m", c=2),
                            )

                def all_to_all(b):
                    nc.gpsimd.collective_compute(
                        "AllToAll",
                        mybir.AluOpType.bypass,
                        replica_groups=[list(range(NCORES))],
                        ins=[a2a_in[b][:, :, :].opt()],
                        outs=[a2a_out[b][:, :, :].opt()],
                    )

                # order: batch-1 attention (and its collective) are emitted
                # before stage3(0) so no engine queue stalls on a2a#0.
                attention(0)
                all_to_all(0)
                attention(1)
                all_to_all(1)
                stage3(0)
                stage3(1)

    nc.compile()
    return nc


def _host_inputs(x, wq_amp, wk_amp, wq_phi, wk_phi, wv, wo):
    """Per-core input maps: pure layout transforms + dtype casts."""
    xT = np.ascontiguousarray(
        x.reshape(N, D).T).astype(BF16)
    woT = np.ascontiguousarray(wo.T).reshape(8, 128, D).astype(BF16)
    # causal 0/1 masks for the 4 diagonal k-tile offsets
    kl = np.arange(128)[:, None]
    ql = np.arange(512)[None, :]
    masks = np.stack(
        [(ql >= kl + 128 * p) for p in range(4)]).astype(BF16)
    in_maps = []
    for i in range(NCORES):
        rsl = slice(DPC * i, DPC * (i + 1))
        in_maps.append({
            "xT": xT,
            "wqaT": np.ascontiguousarray(wq_amp[rsl].T).astype(BF16),
            "wkaT": np.ascontiguousarray(wk_amp[rsl].T).astype(BF16),
            "wqpT": np.ascontiguousarray(wq_phi[rsl].T).astype(BF16),
            "wkpT": np.ascontiguousarray(wk_phi[rsl].T).astype(BF16),
            "wvT": np.ascontiguousarray(wv[rsl].T).astype(BF16),
            "woT": woT,
            "masks": masks,
        })
    return in_maps


def _assemble(results):
    out = np.empty((B, S, D), np.float32)
    for i, r in enumerate(results):
        yc = r["y"].reshape(B, 256, D)
        for b in range(B):
            out[b, 256 * i:256 * (i + 1), :] = yc[b]
    return out


def run(inputs, trace=False):
    from concourse.bass_utils import run_bass_kernel_spmd

    if "nc" not in _CACHE:
        _CACHE["nc"] = _build_program()
    nc = _CACHE["nc"]
    in_maps = _host_inputs(**inputs)
    res = run_bass_kernel_spmd(
        nc, in_maps, list(range(NCORES)), trace=trace)
    return _assemble(res.results), res


def kernel(x, wq_amp, wk_amp, wq_phi, wk_phi, wv, wo):
    out, _ = run(dict(x=x, wq_amp=wq_amp, wk_amp=wk_amp, wq_phi=wq_phi,
                      wk_phi=wk_phi, wv=wv, wo=wo))
    return out
